# revision 14
# baseline (speedup 1.0000x reference)
"""Trainium2 Bass kernel for nn_EneSc.

reference computation (T=16384, D=4096, QD=256, H=128):
    s        = sum_t E_s[t]                 # [D]
    energy_s = dot(s, s)
    c        = sum_t Att[t] * E_s[t]        # [D]
    energy_c = dot(c, c)
    r        = energy_c / energy_s
    r_th     = sigmoid(W2 @ relu(W1 @ E_q + b1) + b2)
    out      = [r, r_th]

Strategy: data-parallel over T across 8 cores (2048 rows/core). The host
casts E_s to fp8_e4m3 (TRN FP8_EXP4; inputs are N(0,1) so |x| << 240 and
the OCP/TRN encodings agree); the r = energy_c/energy_s ratio cancels
quantization error almost perfectly (measured end-to-end rel err vs the
fp32 reference ~1e-5..6e-5, against a 2e-2 gate). This quarters the HBM
stream to 8 MiB/core, which fits in SBUF entirely (64 KiB/partition).

Each core streams 8 pair-tiles [128, 2, 4096] f8 (one DMA each, 8 KiB
contiguous partition lines; row order is irrelevant to a sum so the
natural row-major layout already gives contiguous lines) and reduces
over rows with TensorE DoubleRow fp8 matmuls: stationary [128, 2, 2]
([ones | w] per k-tile), moving [128, 2, 512], accumulating the two row
sub-blocks per pass into fp32 PSUM (64 matmuls, ~15us, hidden under the
~20us stream). The last 128 rows are host-relayouted so the tail
column-split pieces (256/192/64 KiB) are DRAM-contiguous; their closing
matmuls + PSUM drains + bf16 stores chase the stream piecewise. Each
dma_start costs ~0.6-0.8us of HWDGE descriptor-gen on its ring and the
ring only runs a few DMAs ahead, so the DMA count is kept minimal and
the [ones|w] loads ride the otherwise-idle scalar (ACT) HWDGE ring --
which also warms that ring so the final tiny store's descriptor-gen can
overlap the sync ring's. PE is pre-warmed with dummy matmuls through the
preamble (HAM clock gate) into a bank the real accumulation later resets
with start=True. All stream DMA is HWDGE on the sync ring (SWDGE/gpsimd
caused a persistent periodic DMA-engine-15 degradation). Host sums the
8 partial [2, 4096] bf16 outputs in fp64 and runs the tiny MLP.
"""

import ml_dtypes
import numpy as np

from concourse import bacc, mybir, tile
from concourse.bass_utils import run_bass_kernel_spmd

T, D = 16384, 4096
NCORES = 8
RPC = T // NCORES          # rows per core = 2048
P = 128                    # SBUF partitions
KP = 120                   # tail-tile partitions (ports 13/15 relieved)
NPAIR = 8                  # DoubleRow sub-block pairs (256 rows each)
CHUNK = 512                # matmul free-dim (one PSUM bank of fp32)
NCHUNK = D // CHUNK        # 8
# tail column split points of the last 128 rows (DRAM-contiguous pieces)
SPLITS = [(0, 2048), (2048, 3584), (3584, 4096)]

_cached = {}


def _build():
    nc = bacc.Bacc("TRN2", debug=False, num_devices=NCORES)
    f32 = mybir.dt.float32
    bf16 = mybir.dt.bfloat16
    f8 = mybir.dt.float8e4

    e = nc.dram_tensor("e", [RPC * D], f8, kind="ExternalInput")
    # host-prebuilt stationary operands [128, 2, NPAIR, 2]:
    # [p, i, q, :] = [1.0, w(row held by partition p, pair q, k-tile i)].
    # k-tile is dim 1 so its stride is 16 B -- the dual-fp8 LDWEIGHTS
    # verifier (s3_lw_dual_fp8_restrictions) requires outer free strides
    # to be even multiples of 16 B.
    lw = nc.dram_tensor("lw", [P, 2, NPAIR, 2], f8, kind="ExternalInput")
    lwx = nc.dram_tensor("lwx", [16, 2], f8, kind="ExternalInput")
    # bf16 output: halves the tail copy+store chain; the [2,4096]
    # partials are ~|100| so bf16 adds only ~1e-5 end-to-end error
    o = nc.dram_tensor("o", [2, D], bf16, kind="ExternalOutput")

    e_flat = e.ap()
    PAIR = 2 * P * D            # elements per pair-tile (256 rows)

    with tile.TileContext(nc) as tc:
        with (
            tc.tile_pool(name="const", bufs=1) as const,
            tc.tile_pool(name="psum", bufs=1, space="PSUM") as psum,
            tc.tile_pool(name="data", bufs=NPAIR) as data,
            tc.tile_pool(name="out", bufs=1) as outp,
        ):
            # ---- stream DMAs; the whole 8 MiB shard fits in SBUF ----
            tiles = []
            for q in range(NPAIR - 1):   # pairs 0..6: rows 256q..256q+255
                t = data.tile([P, 2, D], f8, name=f"t{q}", tag="data")
                nc.sync.dma_start(
                    t[:],
                    e_flat[q * PAIR : (q + 1) * PAIR].rearrange(
                        "(p h) -> p h", p=P
                    ),
                )
                tiles.append(t)
                if q == 0:
                    # stationary operands ride the idle ACT HWDGE ring:
                    # keeps the sync ring's descriptor-gen for the data
                    # stream, and warms the ACT ring for the last store
                    lhs = const.tile([P, 2, NPAIR, 2], f8)
                    nc.scalar.dma_start(lhs[:], lw.ap())
                    lhsx = const.tile([16, 2], f8)
                    nc.scalar.dma_start(lhsx[:], lwx.ap())
                    # side tile X: the 16 rows displaced from partitions
                    # 120..127 of the tail tile (engine-15 relief); its
                    # 64 KiB also ride the idle ACT ring
                    tx = data.tile([16, D], f8, name="tx", tag="data")
                    x_off = RPC * D - 16 * D
                    nc.scalar.dma_start(
                        tx[:],
                        e_flat[x_off : x_off + 16 * D].rearrange(
                            "(p h) -> p h", p=16
                        ),
                    )
            # pair 7 (tail): partitions 0..119 only -- profiler traffic
            # periodically steals ~10-15% of DMA engine 15, so ports
            # 13/15 carry ~12.5% fewer bytes and absorb the theft
            # instead of straggling the stream end. k-tile 0 = rows
            # 1792..1911 full-D; k-tile 1 = rows 1920..2039 column-split
            # into host-relayouted contiguous pieces.
            t7 = data.tile([P, 2, D], f8, name="t7", tag="data")
            off = 7 * PAIR
            nc.sync.dma_start(
                t7[0:KP, 0, :],
                e_flat[off : off + KP * D].rearrange("(p h) -> p h", p=KP),
            )
            off += KP * D
            # k-tile 1 as ONE DMA: each dma_start costs ~0.8us of
            # ring-serialized descriptor-gen, which exceeds the stream
            # time of small pieces -- the old 3-piece chase was
            # descriptor-bound and the 8 closing matmuls (0.23us each)
            # dominate the tail regardless.
            nc.sync.dma_start(
                t7[0:KP, 1, :],
                e_flat[off : off + KP * D].rearrange("(p h) -> p h", p=KP),
            )
            off += KP * D
            tiles.append(t7)

            acc = [
                psum.tile([2, CHUNK], f32, name=f"acc{c}", tag=f"acc{c}")
                for c in range(NCHUNK)
            ]
            o_sb = outp.tile([2, D], bf16)

            # ---- PE warm-up: HAM gates the PE clock to 1.2 GHz until it
            # sees ~3.4us of sustained activity; real matmuls start only
            # once tile 0 lands (~12us in). Dummy matmuls on a memset
            # scratch tile keep PE busy through the preamble; the group
            # is closed and the real accumulation resets the bank with
            # start=True, so the garbage never escapes.
            scratch = const.tile([P, 320], f8)
            nc.vector.memset(scratch[:], 1.0)
            NWARM = 16
            for k in range(NWARM):
                nc.tensor.matmul(
                    acc[0][:, 0:320],
                    scratch[:, 0:2],
                    scratch[:],
                    start=(k == 0),
                    stop=(k == NWARM - 1),
                )

            # ---- DoubleRow matmuls: 8 pairs x 8 chunks into 8 PSUM banks
            # (the closing pair contracts K=120; the 16 displaced rows are
            # added by plain K=16 matmuls on the side tile after pair 0)
            for q in range(NPAIR):
                last = q == NPAIR - 1
                kp = KP if last else P
                for c in range(NCHUNK):
                    nc.tensor.matmul(
                        acc[c][:],
                        lhs[0:kp, :, q, :],
                        tiles[q][0:kp, :, c * CHUNK : (c + 1) * CHUNK],
                        start=(q == 0),
                        stop=last,
                        perf_mode=mybir.MatmulPerfMode.DoubleRow,
                    )
                if q == 0:
                    for c in range(NCHUNK):
                        nc.tensor.matmul(
                            acc[c][:],
                            lhsx[:],
                            tx[:, c * CHUNK : (c + 1) * CHUNK],
                            start=False,
                            stop=False,
                        )
                    if last:
                        # drain each chunk as its group closes; alternate
                        # DVE / ACT so the copies pipeline. Stores chase
                        # the tail pieces; the final tiny store's
                        # descriptor-gen rides the (warmed) ACT ring in
                        # parallel with the sync ring's second store.
                        lo, hi = c * CHUNK, (c + 1) * CHUNK
                        if c % 2 == 0:
                            nc.vector.tensor_copy(o_sb[:, lo:hi], acc[c][:])
                        else:
                            nc.scalar.copy(o_sb[:, lo:hi], acc[c][:])
                        if c == 3:
                            nc.sync.dma_start(o.ap()[:, :2048], o_sb[:, :2048])
                        elif c == 6:
                            nc.sync.dma_start(
                                o.ap()[:, 2048:3584], o_sb[:, 2048:3584]
                            )
                        elif c == 7:
                            nc.scalar.dma_start(o.ap()[:, 3584:], o_sb[:, 3584:])

    nc.compile()
    return nc


def _get_nc():
    if "nc" not in _cached:
        _cached["nc"] = _build()
    return _cached["nc"]


def _prep_shard(shard, w):
    """Cast to fp8_e4m3 and lay out for the kernel. Rows 0..1919 keep the
    natural row-major order (pair-tile q, partition p, k-tile i holds row
    256q + 2p + i; pair 7 k-tile 0 holds row 1792 + p). The last 128 rows
    (1920 + p) are relayouted so each SPLITS column-piece is contiguous.
    Returns (e_dev flat fp8, lw [P, 2, NPAIR, 2] fp8)."""
    q8 = shard.astype(ml_dtypes.float8_e4m3)
    parts = [q8[:1792].reshape(-1), q8[1792 : 1792 + KP].reshape(-1)]
    parts.append(q8[1920 : 1920 + KP].reshape(-1))   # tail k-tile 1
    parts.append(q8[1792 + KP : 1920].reshape(-1))   # displaced k0 rows
    parts.append(q8[1920 + KP :].reshape(-1))        # displaced k1 rows
    dev = np.concatenate(parts)
    assert dev.size == RPC * D
    lw = np.empty((P, 2, NPAIR, 2), dtype=np.float32)
    lw[..., 0] = 1.0
    p = np.arange(P)
    for q in range(NPAIR - 1):
        for i in range(2):
            lw[:, i, q, 1] = w[256 * q + 2 * p + i]
    lw[:, :, 7, :] = 0.0                      # K=120 skips partitions 120+
    lw[:KP, 0, 7, 0] = 1.0
    lw[:KP, 0, 7, 1] = w[1792 + p[:KP]]
    lw[:KP, 1, 7, 0] = 1.0
    lw[:KP, 1, 7, 1] = w[1920 + p[:KP]]
    lwx = np.empty((16, 2), dtype=np.float32)
    lwx[:, 0] = 1.0
    lwx[:8, 1] = w[1792 + KP : 1920]
    lwx[8:, 1] = w[1920 + KP :]
    return (
        dev,
        lw.astype(ml_dtypes.float8_e4m3),
        lwx.astype(ml_dtypes.float8_e4m3),
    )


def _run_device(E_s, Att_weights, **spmd_kwargs):
    nc = _get_nc()
    E_s = np.ascontiguousarray(E_s, dtype=np.float32)
    Att = np.ascontiguousarray(Att_weights, dtype=np.float32)
    in_maps = []
    for i in range(NCORES):
        dev, lw, lwx = _prep_shard(
            E_s[i * RPC : (i + 1) * RPC], Att[i * RPC : (i + 1) * RPC]
        )
        in_maps.append({"e": dev, "lw": lw, "lwx": lwx})
    res = run_bass_kernel_spmd(nc, in_maps, core_ids=list(range(NCORES)), **spmd_kwargs)
    partials = np.stack(
        [np.asarray(res.results[i]["o"], dtype=np.float32) for i in range(NCORES)]
    )  # [8, 2, D]
    return partials, res


def kernel(E_s, E_q, Att_weights, W1, b1, W2, b2):
    partials, _ = _run_device(E_s, Att_weights)
    s = partials[:, 0, :].astype(np.float64).sum(axis=0)
    c = partials[:, 1, :].astype(np.float64).sum(axis=0)
    energy_s = float(np.dot(s, s))
    energy_c = float(np.dot(c, c))
    r = energy_c / energy_s
    # tiny replicated MLP on E_q (host, ~70k flops)
    h = np.maximum(W1.astype(np.float64) @ E_q.astype(np.float64) + b1, 0.0)
    z = float((W2.astype(np.float64) @ h)[0] + b2[0])
    r_th = 1.0 / (1.0 + np.exp(-z))
    return np.array([r, r_th], dtype=np.float32)


# revision 15
# speedup vs baseline: 1.0188x; 1.0188x over previous
"""Trainium2 Bass kernel for nn_EneSc.

reference computation (T=16384, D=4096, QD=256, H=128):
    s        = sum_t E_s[t]                 # [D]
    energy_s = dot(s, s)
    c        = sum_t Att[t] * E_s[t]        # [D]
    energy_c = dot(c, c)
    r        = energy_c / energy_s
    r_th     = sigmoid(W2 @ relu(W1 @ E_q + b1) + b2)
    out      = [r, r_th]

Strategy: data-parallel over T across 8 cores (2048 rows/core). The host
casts E_s to fp8_e4m3 (TRN FP8_EXP4; inputs are N(0,1) so |x| << 240 and
the OCP/TRN encodings agree); the r = energy_c/energy_s ratio cancels
quantization error almost perfectly (measured end-to-end rel err vs the
fp32 reference ~1e-5..6e-5, against a 2e-2 gate). This quarters the HBM
stream to 8 MiB/core, which fits in SBUF entirely (64 KiB/partition).

Each core streams 8 pair-tiles [128, 2, 4096] f8 (one DMA each, 8 KiB
contiguous partition lines; row order is irrelevant to a sum so the
natural row-major layout already gives contiguous lines) and reduces
over rows with TensorE DoubleRow fp8 matmuls: stationary [128, 2, 2]
([ones | w] per k-tile), moving [128, 2, 512], accumulating the two row
sub-blocks per pass into fp32 PSUM (64 matmuls, ~15us, hidden under the
~20us stream). Each dma_start costs
~0.6-0.8us of HWDGE descriptor-gen serialized on its ring and the ring
only runs a few DMAs ahead, so the DMA count is kept minimal (9 stream
DMAs on the sync ring) and the [ones|w] loads + side tile ride the
otherwise-idle scalar (ACT) HWDGE ring -- which also warms that ring so
the final tiny store's descriptor-gen can overlap the sync ring's. The
closing matmuls + PSUM drains + bf16 stores chase the final DMAs. PE is pre-warmed with dummy matmuls through the
preamble (HAM clock gate) into a bank the real accumulation later resets
with start=True. All stream DMA is HWDGE on the sync ring (SWDGE/gpsimd
caused a persistent periodic DMA-engine-15 degradation). Host sums the
8 partial [2, 4096] bf16 outputs in fp64 and runs the tiny MLP.
"""

import ml_dtypes
import numpy as np

from concourse import bacc, mybir, tile
from concourse.bass_utils import run_bass_kernel_spmd

T, D = 16384, 4096
NCORES = 8
RPC = T // NCORES          # rows per core = 2048
P = 128                    # SBUF partitions
KP = 120                   # tail-tile partitions (ports 13/15 relieved)
NPAIR = 8                  # DoubleRow sub-block pairs (256 rows each)
CHUNK = 512                # matmul free-dim (one PSUM bank of fp32)
NCHUNK = D // CHUNK        # 8

_cached = {}


def _build():
    nc = bacc.Bacc("TRN2", debug=False, num_devices=NCORES)
    f32 = mybir.dt.float32
    bf16 = mybir.dt.bfloat16
    f8 = mybir.dt.float8e4

    e = nc.dram_tensor("e", [RPC * D], f8, kind="ExternalInput")
    # host-prebuilt stationary operands [128, 2, NPAIR, 2]:
    # [p, i, q, :] = [1.0, w(row held by partition p, pair q, k-tile i)].
    # k-tile is dim 1 so its stride is 16 B -- the dual-fp8 LDWEIGHTS
    # verifier (s3_lw_dual_fp8_restrictions) requires outer free strides
    # to be even multiples of 16 B.
    lw = nc.dram_tensor("lw", [P, 2, NPAIR, 2], f8, kind="ExternalInput")
    lwx = nc.dram_tensor("lwx", [16, 2], f8, kind="ExternalInput")
    # bf16 output: halves the tail copy+store chain; the [2,4096]
    # partials are ~|100| so bf16 adds only ~1e-5 end-to-end error
    o = nc.dram_tensor("o", [2, D], bf16, kind="ExternalOutput")

    e_flat = e.ap()
    PAIR = 2 * P * D            # elements per pair-tile (256 rows)

    with tile.TileContext(nc) as tc:
        with (
            tc.tile_pool(name="const", bufs=1) as const,
            tc.tile_pool(name="psum", bufs=1, space="PSUM") as psum,
            tc.tile_pool(name="data", bufs=NPAIR) as data,
            tc.tile_pool(name="out", bufs=1) as outp,
        ):
            # ---- stream DMAs; the whole 8 MiB shard fits in SBUF ----
            tiles = []
            for q in range(NPAIR - 1):   # pairs 0..6: rows 256q..256q+255
                t = data.tile([P, 2, D], f8, name=f"t{q}", tag="data")
                nc.sync.dma_start(
                    t[:],
                    e_flat[q * PAIR : (q + 1) * PAIR].rearrange(
                        "(p h) -> p h", p=P
                    ),
                )
                tiles.append(t)
                if q == 0:
                    # stationary operands ride the idle ACT HWDGE ring:
                    # keeps the sync ring's descriptor-gen for the data
                    # stream, and warms the ACT ring for the last store
                    lhs = const.tile([P, 2, NPAIR, 2], f8)
                    nc.scalar.dma_start(lhs[:], lw.ap())
                    lhsx = const.tile([16, 2], f8)
                    nc.scalar.dma_start(lhsx[:], lwx.ap())
                    # side tile X: the 16 rows displaced from partitions
                    # 120..127 of the tail tile (engine-15 relief); its
                    # 64 KiB also ride the idle ACT ring
                    tx = data.tile([16, D], f8, name="tx", tag="data")
                    x_off = RPC * D - 16 * D
                    nc.scalar.dma_start(
                        tx[:],
                        e_flat[x_off : x_off + 16 * D].rearrange(
                            "(p h) -> p h", p=16
                        ),
                    )
            # pair 7 (tail): partitions 0..119 only -- profiler traffic
            # periodically steals ~10-15% of DMA engine 15, so ports
            # 13/15 carry ~12.5% fewer bytes and absorb the theft
            # instead of straggling the stream end. k-tile 0 = rows
            # 1792..1911 full-D; k-tile 1 = rows 1920..2039 column-split
            # into host-relayouted contiguous pieces.
            t7 = data.tile([P, 2, D], f8, name="t7", tag="data")
            off = 7 * PAIR
            nc.sync.dma_start(
                t7[0:KP, 0, :],
                e_flat[off : off + KP * D].rearrange("(p h) -> p h", p=KP),
            )
            off += KP * D
            # k-tile 1 as ONE DMA: each dma_start costs ~0.8us of
            # ring-serialized descriptor-gen, which exceeds the stream
            # time of small pieces -- the old 3-piece chase was
            # descriptor-bound and the 8 closing matmuls (0.23us each)
            # dominate the tail regardless.
            nc.sync.dma_start(
                t7[0:KP, 1, :],
                e_flat[off : off + KP * D].rearrange("(p h) -> p h", p=KP),
            )
            off += KP * D
            tiles.append(t7)

            acc = [
                psum.tile([2, CHUNK], f32, name=f"acc{c}", tag=f"acc{c}")
                for c in range(NCHUNK)
            ]
            o_sb = outp.tile([2, D], bf16)

            # ---- PE warm-up: HAM gates the PE clock to 1.2 GHz until it
            # sees ~3.4us of sustained activity; real matmuls start only
            # once tile 0 lands (~12us in). Dummy matmuls on a memset
            # scratch tile keep PE busy through the preamble; the group
            # is closed and the real accumulation resets the bank with
            # start=True, so the garbage never escapes.
            scratch = const.tile([P, 320], f8)
            nc.vector.memset(scratch[:], 1.0)
            NWARM = 16
            for k in range(NWARM):
                nc.tensor.matmul(
                    acc[0][:, 0:320],
                    scratch[:, 0:2],
                    scratch[:],
                    start=(k == 0),
                    stop=(k == NWARM - 1),
                )

            # ---- DoubleRow matmuls: 8 pairs x 8 chunks into 8 PSUM banks
            # (the closing pair contracts K=120; the 16 displaced rows are
            # added by plain K=16 matmuls on the side tile after pair 0)
            for q in range(NPAIR):
                last = q == NPAIR - 1
                kp = KP if last else P
                for c in range(NCHUNK):
                    nc.tensor.matmul(
                        acc[c][:],
                        lhs[0:kp, :, q, :],
                        tiles[q][0:kp, :, c * CHUNK : (c + 1) * CHUNK],
                        start=(q == 0),
                        stop=last,
                        perf_mode=mybir.MatmulPerfMode.DoubleRow,
                    )
                if q == 0:
                    for c in range(NCHUNK):
                        nc.tensor.matmul(
                            acc[c][:],
                            lhsx[:],
                            tx[:, c * CHUNK : (c + 1) * CHUNK],
                            start=False,
                            stop=False,
                        )
                    if last:
                        # drain each chunk as its group closes; alternate
                        # DVE / ACT so the copies pipeline. Stores chase
                        # the tail pieces; the final tiny store's
                        # descriptor-gen rides the (warmed) ACT ring in
                        # parallel with the sync ring's second store.
                        lo, hi = c * CHUNK, (c + 1) * CHUNK
                        if c % 2 == 0:
                            nc.vector.tensor_copy(o_sb[:, lo:hi], acc[c][:])
                        else:
                            nc.scalar.copy(o_sb[:, lo:hi], acc[c][:])
                        if c == 3:
                            nc.sync.dma_start(o.ap()[:, :2048], o_sb[:, :2048])
                        elif c == 6:
                            nc.sync.dma_start(
                                o.ap()[:, 2048:3584], o_sb[:, 2048:3584]
                            )
                        elif c == 7:
                            nc.scalar.dma_start(o.ap()[:, 3584:], o_sb[:, 3584:])

    nc.compile()
    return nc


def _get_nc():
    if "nc" not in _cached:
        _cached["nc"] = _build()
    return _cached["nc"]


def _prep_shard(shard, w):
    """Cast to fp8_e4m3 and lay out for the kernel.

    Device flat layout (f8): rows 0..1791 natural row-major (pair-tile q,
    partition p, k-tile i holds row 256q + 2p + i), tail k-tile 0 = rows
    1792..1911 (partition p holds row 1792+p, p<120), tail k-tile 1 =
    rows 1920..2039, then the 16 displaced rows {1912..1919, 2040..2047}
    as the side tile X. Weight slots mirror the row placement."""
    q8 = shard.astype(ml_dtypes.float8_e4m3)
    parts = [q8[:1792].reshape(-1), q8[1792 : 1792 + KP].reshape(-1)]
    parts.append(q8[1920 : 1920 + KP].reshape(-1))   # tail k-tile 1
    parts.append(q8[1792 + KP : 1920].reshape(-1))   # displaced k0 rows
    parts.append(q8[1920 + KP :].reshape(-1))        # displaced k1 rows
    dev = np.concatenate(parts)
    assert dev.size == RPC * D
    lw = np.empty((P, 2, NPAIR, 2), dtype=np.float32)
    lw[..., 0] = 1.0
    p = np.arange(P)
    for q in range(NPAIR - 1):
        for i in range(2):
            lw[:, i, q, 1] = w[256 * q + 2 * p + i]
    lw[:, :, 7, :] = 0.0                      # K=120 skips partitions 120+
    lw[:KP, 0, 7, 0] = 1.0
    lw[:KP, 0, 7, 1] = w[1792 + p[:KP]]
    lw[:KP, 1, 7, 0] = 1.0
    lw[:KP, 1, 7, 1] = w[1920 + p[:KP]]
    lwx = np.empty((16, 2), dtype=np.float32)
    lwx[:, 0] = 1.0
    lwx[:8, 1] = w[1792 + KP : 1920]
    lwx[8:, 1] = w[1920 + KP :]
    return (
        dev,
        lw.astype(ml_dtypes.float8_e4m3),
        lwx.astype(ml_dtypes.float8_e4m3),
    )


def _run_device(E_s, Att_weights, **spmd_kwargs):
    nc = _get_nc()
    E_s = np.ascontiguousarray(E_s, dtype=np.float32)
    Att = np.ascontiguousarray(Att_weights, dtype=np.float32)
    in_maps = []
    for i in range(NCORES):
        dev, lw, lwx = _prep_shard(
            E_s[i * RPC : (i + 1) * RPC], Att[i * RPC : (i + 1) * RPC]
        )
        in_maps.append({"e": dev, "lw": lw, "lwx": lwx})
    res = run_bass_kernel_spmd(nc, in_maps, core_ids=list(range(NCORES)), **spmd_kwargs)
    partials = np.stack(
        [np.asarray(res.results[i]["o"], dtype=np.float32) for i in range(NCORES)]
    )  # [8, 2, D]
    return partials, res


def kernel(E_s, E_q, Att_weights, W1, b1, W2, b2):
    partials, _ = _run_device(E_s, Att_weights)
    s = partials[:, 0, :].astype(np.float64).sum(axis=0)
    c = partials[:, 1, :].astype(np.float64).sum(axis=0)
    energy_s = float(np.dot(s, s))
    energy_c = float(np.dot(c, c))
    r = energy_c / energy_s
    # tiny replicated MLP on E_q (host, ~70k flops)
    h = np.maximum(W1.astype(np.float64) @ E_q.astype(np.float64) + b1, 0.0)
    z = float((W2.astype(np.float64) @ h)[0] + b2[0])
    r_th = 1.0 / (1.0 + np.exp(-z))
    return np.array([r, r_th], dtype=np.float32)
